# revision 55
# baseline (speedup 1.0000x reference)
"""BiRWKV layer kernel for Trainium2 (8 NeuronCores, Bass/Tile).

Problem: x[4,2048,1024] -> rkv = x @ rkv_w.T -> (r,k,v) fwd + bwd,
WKV scan per direction, gate with sigmoid(r), concat, out @ out_w.T.

Strategy (v2):
  - Shard over (batch b, channel-half h): core = 2*b + h. Each core handles
    one batch's 512 fwd + 512 bwd channels end-to-end.
  - Channels-on-partitions layout [c, t]: projections via PE matmul
    (lhsT = W^T tile [c,d], rhs = x^T [c,t]), WKV recurrence via the DVE's
    native tensor_tensor_scan, out-projection partial via PE (contraction
    over local c), summed across cores on host (bf16 partials).
  - k,v matmuls in bf16; r matmuls in fp8e4 DoubleRow mode (2x PE rate,
    K=256 per instruction). r only passes through sigmoid (bounded
    derivative), so fp8 error stays within the rel-err budget. Wr is
    pre-scaled x32 on host; descale folded into the tanh activation scale.
  - WKV scan runs unstabilized in fp32: A_t = lam*A + e^k v, D_t = lam*D
    + e^k. Scans are reformulated as the SHIFTED recurrence
    S_t = lam*S_{t-1} + u_{t-1} (u = pv or p, with the chunk carry riding
    in slot 0 of a [128, TCH+1] input tile), so S_t = A_{t-1} lands in an
    ALIGNED [128, TCH] output that num/den read directly:
       num = eu*pv + S_A,  den = eu*p + S_D  (stt ops)
    The next chunk's carry A_TCH = lam*S_TCH + u_TCH is a tiny [128,1]
    stt on gpsimd into the next chunk's input slot 0.
  - sigmoid(r)*y = 0.5*(1+tanh(r/2))*y; the 0.5 is folded into out_w.
  - Engine placement per chain: ACT exp/tanh/v-drain + most output drains;
    DVE pv, both scans, num/den, reciprocal; gpsimd y, z, carries. Stages
    are software-pipelined with lags (rec/y lag one chain, z two) so no
    strict-FIFO queue blocks at its head.
  - Tail: the last chunk's out-projection is emitted in K-split waves
    (ct01 -> ct2 -> ct3) so the PE keeps running while the final z tiles
    come down the vector pipeline.
"""
import os
import sys
import numpy as np

sys.path.insert(0, "/opt/trn_rl_repo")

import ml_dtypes

import concourse.bass as bass
import concourse.mybir as mybir
from concourse import bacc
import concourse.tile as tile
from concourse.bass_utils import run_bass_kernel_spmd

B, T, C = 4, 2048, 1024
H = C // 2          # channels per core per direction (512)
NCT = H // 128      # c-tiles per direction (4)
TCH = 512           # time chunk
NTC = T // TCH      # t-chunks (4)
PAD = 15            # carry slot index in u-tiles; keeps p/pv reads 32B-aligned
F32 = mybir.dt.float32
BF16 = mybir.dt.bfloat16
FP8 = mybir.dt.float8e4
AF = mybir.ActivationFunctionType
ALU = mybir.AluOpType
PM = mybir.MatmulPerfMode

R_FP8 = os.environ.get("BIRWKV_R_FP8", "1") == "1"
OUT_BF16 = os.environ.get("BIRWKV_OUT_BF16", "1") == "1"
TAIL_SPLIT = os.environ.get("BIRWKV_TAIL", "1") == "1"
RSW = 32.0           # host pre-scale on Wr when fp8

NP_BF16 = ml_dtypes.bfloat16
NP_FP8 = ml_dtypes.float8_e4m3
OUT_DT = BF16 if OUT_BF16 else F32
NP_OUT = NP_BF16 if OUT_BF16 else np.float32

_compiled = None


def _build():
    nc = bacc.Bacc("TRN2", target_bir_lowering=False, debug=False, num_devices=8)

    # per-core inputs, all partition-major so batched DMAs walk source and
    # dest in the same order: xT[p, ck, t] holds x^T channel c = ck*128+p.
    xT = nc.dram_tensor("xT", [128, 8, T], BF16, kind="ExternalInput").ap()
    if R_FP8:
        xT8 = nc.dram_tensor("xT8", [128, 8, T], FP8, kind="ExternalInput").ap()
    Ws = {}
    for nm in ("Wkf", "Wvf", "Wkb", "Wvb"):
        Ws[nm] = nc.dram_tensor(nm, [128, 8, H], BF16, kind="ExternalInput").ap()
    rdt = FP8 if R_FP8 else BF16
    for nm in ("Wrf", "Wrb"):
        Ws[nm] = nc.dram_tensor(nm, [128, 8, H], rdt, kind="ExternalInput").ap()
    Wof = nc.dram_tensor("Wof", [128, NCT, C], BF16, kind="ExternalInput").ap()
    Wob = nc.dram_tensor("Wob", [128, NCT, C], BF16, kind="ExternalInput").ap()
    lamf = nc.dram_tensor("lamf", [128, NCT], F32, kind="ExternalInput").ap()
    lamb = nc.dram_tensor("lamb", [128, NCT], F32, kind="ExternalInput").ap()
    euf = nc.dram_tensor("euf", [128, NCT], F32, kind="ExternalInput").ap()
    eub = nc.dram_tensor("eub", [128, NCT], F32, kind="ExternalInput").ap()

    # outputs [p, et, t]: channel c = et*128 + p
    outTf = nc.dram_tensor("outTf", [128, 8, T], OUT_DT, kind="ExternalOutput").ap()
    outTb = nc.dram_tensor("outTb", [128, 8, T], OUT_DT, kind="ExternalOutput").ap()

    with tile.TileContext(nc) as tc:
        with (
            tc.tile_pool(name="wk", bufs=1) as wk_pool,
            tc.tile_pool(name="wv", bufs=1) as wv_pool,
            tc.tile_pool(name="wr", bufs=1) as wr_pool,
            tc.tile_pool(name="wo", bufs=1) as wo_pool,
            tc.tile_pool(name="xr", bufs=1) as xr_pool,
            tc.tile_pool(name="lam", bufs=1) as lam_pool,
            tc.tile_pool(name="ew", bufs=2) as ew_pool,
            tc.tile_pool(name="zs", bufs=4) as z_pool,
            tc.tile_pool(name="ab", bufs=2) as ab_pool,
            tc.tile_pool(name="osb", bufs=2) as osb_pool,
            tc.tile_pool(name="pp", bufs=5, space="PSUM") as pp,
            tc.tile_pool(name="po", bufs=3, space="PSUM") as po,
        ):
            # ---- resident tiles; startup-critical DMAs first -----------
            # x chunk 0 + Wkf split per-ck so their semaphores fire
            # progressively and the first matmul chain starts early.
            x_t = {}
            for q in range(8):
                x_t[(0, q)] = xr_pool.tile([128, 1, TCH], BF16,
                                           tag=f"x0q{q}", name=f"x0q{q}")
            for ti in range(1, NTC):
                x_t[ti] = xr_pool.tile([128, 8, TCH], BF16, tag=f"x{ti}",
                                       name=f"x{ti}")
            wk0q = [wk_pool.tile([128, 1, H], BF16, tag=f"wk0q{q}",
                                 name=f"wk0q{q}") for q in range(8)]
            wk_t = {1: wk_pool.tile([128, 8, H], BF16, tag="wk1", name="wk1")}
            wv_t = {d: wv_pool.tile([128, 8, H], BF16, tag=f"wv{d}",
                                    name=f"wv{d}") for d in range(2)}
            wr_t = {d: wr_pool.tile([128, 8, H], rdt, tag=f"wr{d}",
                                    name=f"wr{d}") for d in range(2)}
            wo_t = {d: wo_pool.tile([128, NCT, C], BF16, tag=f"wo{d}",
                                    name=f"wo{d}") for d in range(2)}
            x8_t = {}
            if R_FP8:
                for ti in range(NTC):
                    x8_t[ti] = xr_pool.tile([128, 8, TCH], FP8, tag=f"x8{ti}",
                                            name=f"x8{ti}")
            for q in range(8):
                nc.sync.dma_start(wk0q[q][:], Ws["Wkf"][:, q:q + 1])
                nc.sync.dma_start(x_t[(0, q)][:], xT[:, q:q + 1, 0:TCH])
            nc.sync.dma_start(wv_t[0][:], Ws["Wvf"][:])
            nc.sync.dma_start(wr_t[0][:], Ws["Wrf"][:])
            if R_FP8:
                nc.sync.dma_start(x8_t[0][:], xT8[:, :, 0:TCH])
            nc.sync.dma_start(x_t[1][:], xT[:, :, TCH:2 * TCH])
            lam_t = {}
            eu_t = {}
            for d, (lam_d, eu_d) in enumerate(((lamf, euf), (lamb, eub))):
                lt = lam_pool.tile([128, NCT], F32, tag=f"lam{d}", name=f"lt{d}")
                nc.sync.dma_start(lt[:], lam_d[:])
                et = lam_pool.tile([128, NCT], F32, tag=f"eu{d}", name=f"et{d}")
                nc.sync.dma_start(et[:], eu_d[:])
                lam_t[d] = lt
                eu_t[d] = et

            # deferred input DMAs: issued at chain checkpoints (n -> fn)
            def dma_later():
                if R_FP8:
                    yield lambda: nc.sync.dma_start(x8_t[1][:],
                                                    xT8[:, :, TCH:2 * TCH])
                yield lambda: nc.sync.dma_start(wo_t[0][:], Wof[:])
                yield lambda: nc.sync.dma_start(x_t[2][:],
                                                xT[:, :, 2 * TCH:3 * TCH])
                yield lambda: nc.sync.dma_start(x_t[3][:],
                                                xT[:, :, 3 * TCH:4 * TCH])
                if R_FP8:
                    yield lambda: nc.sync.dma_start(x8_t[2][:],
                                                    xT8[:, :, 2 * TCH:3 * TCH])
                    yield lambda: nc.sync.dma_start(x8_t[3][:],
                                                    xT8[:, :, 3 * TCH:4 * TCH])
                yield lambda: nc.sync.dma_start(wk_t[1][:], Ws["Wkb"][:])
                yield lambda: nc.sync.dma_start(wv_t[1][:], Ws["Wvb"][:])
                yield lambda: nc.sync.dma_start(wr_t[1][:], Ws["Wrb"][:])
                yield lambda: nc.sync.dma_start(wo_t[1][:], Wob[:])
            dma_q = dma_later()

            prevA = {}
            prevD = {}
            records = []
            z_by_chunk = {}
            ostages = {}
            op_q = []             # pending out-projection (chunk, e0, e1)

            def stage1(d, ti, ct, k_ps, v_ps, r_ps):
                """Baseline scan layout: p/pv in aligned full tiles (fast
                stt reads); inclusive scan A_t writes a_buf[:, 1:TCH+1]
                (initial = carry at [:, 0:1]); num/den read [:, 0:TCH]."""
                p = ew_pool.tile([128, TCH], BF16, tag="p", bufs=4, name="p")
                nc.scalar.activation(p[:], k_ps[:], AF.Exp)
                # tanh shares the ACT table set with exp (sigmoid does not);
                # z = (tanh(r/2)+1)*y with the 0.5 folded into Wo. th1 = th+1
                # is a second ACT op (Copy w/ bias) so z is ONE Pool multiply
                # off the latency-critical DVE queue.
                th = ew_pool.tile([128, TCH], BF16, tag="th", bufs=3, name="th")
                th_scale = 0.5 / RSW if R_FP8 else 0.5
                nc.scalar.activation(th[:], r_ps[:], AF.Tanh, scale=th_scale)
                v_sb = ew_pool.tile([128, TCH], BF16, tag="vsb", bufs=4,
                                    name="vsb")
                nc.scalar.copy(v_sb[:], v_ps[:])
                pv = ew_pool.tile([128, TCH], BF16, tag="pv", bufs=2,
                                  name="pv")
                nc.vector.tensor_tensor(pv[:], p[:], v_sb[:], ALU.mult)

                # 520-wide alloc keeps the partition pitch 32B-aligned
                a_buf = ab_pool.tile([128, TCH + 8], F32, tag=f"A{ct}",
                                     name="ab")
                d_buf = ab_pool.tile([128, TCH + 8], F32, tag=f"D{ct}",
                                     name="db")
                if ti == 0:
                    nc.vector.memset(a_buf[:, 0:1], 0.0)
                    nc.vector.memset(d_buf[:, 0:1], 0.0)
                else:
                    nc.gpsimd.tensor_copy(a_buf[:, 0:1],
                                          prevA[ct][:, TCH:TCH + 1])
                    nc.gpsimd.tensor_copy(d_buf[:, 0:1],
                                          prevD[ct][:, TCH:TCH + 1])
                lam_sl = lam_t[d][:, ct:ct + 1].broadcast_to([128, TCH])
                nc.vector.tensor_tensor_scan(
                    a_buf[:, 1:TCH + 1], lam_sl, pv[:],
                    a_buf[:, 0:1], ALU.mult, ALU.add)
                nc.vector.tensor_tensor_scan(
                    d_buf[:, 1:TCH + 1], lam_sl, p[:],
                    d_buf[:, 0:1], ALU.mult, ALU.add)
                prevA[ct] = a_buf
                prevD[ct] = d_buf

                num = ew_pool.tile([128, TCH], F32, tag="num", bufs=3,
                                   name="num")
                nc.vector.scalar_tensor_tensor(
                    num[:], pv[:], eu_t[d][:, ct:ct + 1], a_buf[:, 0:TCH],
                    ALU.mult, ALU.add)
                den = ew_pool.tile([128, TCH], F32, tag="den", bufs=3,
                                   name="den")
                nc.vector.scalar_tensor_tensor(
                    den[:], p[:], eu_t[d][:, ct:ct + 1], d_buf[:, 0:TCH],
                    ALU.mult, ALU.add)
                return {"d": d, "ti": ti, "ct": ct, "num": num, "den": den,
                        "th": th}

            def stage23(r, on_dve=False):
                if "y" in r:
                    return
                rec = ew_pool.tile([128, TCH], F32, tag="rec", bufs=3, name="rc")
                nc.vector.reciprocal_approx_fast(rec[:], r["den"][:])
                y = ew_pool.tile([128, TCH], BF16, tag="y", bufs=4, name="y")
                if on_dve:
                    nc.vector.tensor_mul(y[:], r["num"][:], rec[:])
                else:
                    nc.gpsimd.tensor_mul(y[:], r["num"][:], rec[:])
                r["y"] = y
                # th1 = th+1 emitted here (lag 1): off the exp->pv ACT
                # latency path, still a round ahead of its z consumer
                th1 = ew_pool.tile([128, TCH], BF16, tag="th1", bufs=3,
                                   name="th1")
                nc.scalar.activation(th1[:], r["th"][:], AF.Copy, bias=1.0)
                r["th1"] = th1

            def stage4(r, on_dve=False):
                if r.get("z_done"):
                    return
                r["z_done"] = True
                z = z_pool.tile([128, TCH], BF16, tag=f'z{r["ct"]}', name="z")
                eng = nc.vector if on_dve else nc.gpsimd
                eng.tensor_mul(z[:], r["th1"][:], r["y"][:])
                z_by_chunk[(r["d"], r["ti"])][r["ct"]] = z

            def emit_outproj(chunk, e0, e1, final=False):
                d, ti = chunk
                outT = outTb if d == 1 else outTf
                t0 = ti * TCH
                ci = d * NTC + ti
                if ci not in ostages:
                    ostages[ci] = osb_pool.tile([128, 8, TCH], OUT_DT,
                                                tag="ost", name="ost")
                ostage = ostages[ci]
                z_tiles = z_by_chunk[chunk]
                for et in range(e0, e1):
                    esl = slice(et * 128, (et + 1) * 128)
                    o_ps = po.tile([128, TCH], F32, tag="ops", name="op")
                    for ct in range(NCT):
                        nc.tensor.matmul(
                            o_ps[:],
                            wo_t[d][:, ct, esl],
                            z_tiles[ct][:],
                            start=(ct == 0), stop=(ct == NCT - 1),
                        )
                    if et % 2 == 1 and final:
                        nc.vector.tensor_copy(ostage[:, et], o_ps[:])
                    else:
                        nc.scalar.copy(ostage[:, et], o_ps[:])
                    if final:
                        nc.sync.dma_start(outT[:, et, t0:t0 + TCH],
                                          ostage[:, et])
                    elif et == 7:
                        nc.sync.dma_start(outT[:, :, t0:t0 + TCH], ostage[:])

            def emit_outproj_tail(chunk):
                """Final chunk out-proj in K-split waves: ct01 for all ets
                first (z2/z3 still in flight), then ct2, then ct3+stop."""
                d, ti = chunk
                outT = outTb if d == 1 else outTf
                t0 = ti * TCH
                ostage = osb_pool.tile([128, 8, TCH], OUT_DT, tag="ost",
                                       name="ost")
                z_tiles = z_by_chunk[chunk]
                NP = 5  # ets done via partial waves (pp ring); rest full
                parts = []
                for et in range(NP):
                    o_ps = pp.tile([128, TCH], F32, tag="proj", name="pt")
                    for ct in (0, 1):
                        nc.tensor.matmul(
                            o_ps[:], wo_t[d][:, ct, et * 128:(et + 1) * 128],
                            z_tiles[ct][:], start=(ct == 0), stop=False,
                            skip_group_check=True)
                    parts.append(o_ps)
                for et in range(NP):
                    nc.tensor.matmul(
                        parts[et][:], wo_t[d][:, 2, et * 128:(et + 1) * 128],
                        z_tiles[2][:], start=False, stop=False,
                        skip_group_check=True)
                for et in range(NP):
                    nc.tensor.matmul(
                        parts[et][:], wo_t[d][:, 3, et * 128:(et + 1) * 128],
                        z_tiles[3][:], start=False, stop=True,
                        skip_group_check=True)
                    # spread tail drains across ACT/DVE (gpsimd can't read
                    # PSUM)
                    if et % 2 == 1:
                        nc.vector.tensor_copy(ostage[:, et], parts[et][:])
                    else:
                        nc.scalar.copy(ostage[:, et], parts[et][:])
                    nc.sync.dma_start(outT[:, et, t0:t0 + TCH], ostage[:, et])
                for et in range(NP, 8):
                    o_ps = po.tile([128, TCH], F32, tag="ops", name="op")
                    for ct in range(NCT):
                        nc.tensor.matmul(
                            o_ps[:], wo_t[d][:, ct, et * 128:(et + 1) * 128],
                            z_tiles[ct][:],
                            start=(ct == 0), stop=(ct == NCT - 1))
                    if et % 2 == 1:
                        nc.vector.tensor_copy(ostage[:, et], o_ps[:])
                    else:
                        nc.scalar.copy(ostage[:, et], o_ps[:])
                    nc.sync.dma_start(outT[:, et, t0:t0 + TCH], ostage[:, et])

            def wsl(d, w, ck, dsl):
                if w == "k" and d == 0:
                    return wk0q[ck][:, 0, dsl]
                wmap = {"k": wk_t.get(d), "v": wv_t[d], "r": wr_t[d]}
                return wmap[w][:, ck, dsl]

            def xsl(tis, ck, rev):
                if tis == 0:
                    s = x_t[(0, ck)][:, 0]
                else:
                    s = x_t[tis][:, ck]
                return s[:, ::-1] if rev else s

            n = 0
            for d in range(2):
                rev = (d == 1)
                prevA.clear()
                prevD.clear()
                for ti in range(NTC):
                    z_by_chunk[(d, ti)] = [None] * NCT
                    tis = NTC - 1 - ti if rev else ti
                    # chunk 0: lead with k-sweeps (weights stream in per-ck)
                    # but keep <=5 live psum tiles (pp ring is 5)
                    order = ([("k", 0), ("k", 1), ("k", 2), ("v", 0),
                              ("r", 0), ("k", 3), ("v", 1), ("r", 1),
                              ("v", 2), ("r", 2), ("v", 3), ("r", 3)]) \
                        if (d == 0 and ti == 0) else \
                        [(w, ct) for ct in range(NCT) for w in ("k", "v", "r")]
                    kvr_ps = {}
                    for w, ct in order:
                        dsl = slice(ct * 128, (ct + 1) * 128)
                        ps = pp.tile([128, TCH], F32, tag="proj", name="ps")
                        if w == "r" and R_FP8:
                            for qq in range(4):
                                x8s = x8_t[tis][:, 2 * qq:2 * qq + 2]
                                if rev:
                                    x8s = x8s[:, :, ::-1]
                                nc.tensor.matmul(
                                    ps[:], wr_t[d][:, 2 * qq:2 * qq + 2, dsl],
                                    x8s,
                                    start=(qq == 0), stop=(qq == 3),
                                    perf_mode=PM.DoubleRow,
                                )
                        else:
                            for ck in range(8):
                                nc.tensor.matmul(
                                    ps[:], wsl(d, w, ck, dsl),
                                    xsl(tis, ck, rev),
                                    start=(ck == 0), stop=(ck == 7),
                                )
                        kvr_ps[(w, ct)] = ps
                        if w != "r":
                            continue
                        # deferred DMA issue, spread across early chains
                        if n < 10:
                            fn = next(dma_q, None)
                            if fn is not None:
                                fn()
                        # out-projection pop FIRST: its psum drain lands in
                        # the ACT queue ahead of this chain's 4 ACT ops.
                        # Only pop once all four z tiles exist (the (1,2)
                        # boundary round would otherwise race the collapse).
                        if n >= 10 and op_q:
                            cj, e0, e1, mn = op_q[0]
                            if n >= mn and all(
                                    zt is not None for zt in z_by_chunk[cj]):
                                op_q.pop(0)
                                emit_outproj(cj, e0, e1)
                        # staged vchain emission: consumers of last-round
                        # outputs first so no FIFO blocks at its head
                        if n >= 1:
                            stage23(records[n - 1])
                        if n >= 2:
                            stage4(records[n - 2])
                        records.append(stage1(d, ti, ct,
                                              kvr_ps[("k", ct)],
                                              kvr_ps[("v", ct)],
                                              kvr_ps[("r", ct)]))
                        # backward direction: collapse stage lags so z tiles
                        # enqueue as early as possible (Pool is shallow)
                        if d == 1:
                            stage23(records[n])
                            if n >= 1:
                                stage4(records[n - 1])
                        # out-projection: chunk j enqueues its three et
                        # groups once its z tiles are in flight; the PE pops
                        # at most one group per chain starting at chain 10
                        if ct == 3 and not (d == 1 and ti == NTC - 1):
                            cj = records[n - 3]["d"], records[n - 3]["ti"]
                            mb = n - 3 + (7 if d == 0 else 10)
                            op_q.extend([(cj, 0, 2, mb), (cj, 2, 4, mb + 1),
                                         (cj, 4, 8, mb + 2)])
                        n += 1

            # drain: finish pipeline, flush pending emissions, then the
            # final chunk (K-split waves cover its z latency)
            stage23(records[n - 1])
            stage4(records[n - 2])
            stage4(records[n - 1])
            for cj, e0, e1, _mn in op_q:
                emit_outproj(cj, e0, e1)
            if TAIL_SPLIT:
                emit_outproj_tail((1, NTC - 1))
            else:
                emit_outproj((1, NTC - 1), 0, 8, final=True)

    nc.compile()
    return nc


def _prep_inputs(x, rkv_w, out_w, time_decay, time_first, time_decay_rev,
                 time_first_rev):
    """Host-side sharding + layout prep. Returns list of 8 input dicts."""
    f32 = np.float32
    in_maps = []
    wd_f = -np.exp(time_decay.astype(np.float64))
    wd_b = -np.exp(time_decay_rev.astype(np.float64))
    lam_full_f = np.exp(wd_f).astype(f32)        # [C]
    lam_full_b = np.exp(wd_b).astype(f32)
    eu_full_f = np.exp(time_first.astype(np.float64)).astype(f32)
    eu_full_b = np.exp(time_first_rev.astype(np.float64)).astype(f32)

    for core in range(8):
        b, h = core // 2, core % 2
        cs = slice(h * H, h * H + H)
        xb = x[b].T.astype(f32)                                    # [C, T]
        xtile_f = np.ascontiguousarray(
            xb.reshape(8, 128, T).transpose(1, 0, 2))
        def wtile(w, dt):   # [C, H] -> [128, 8, H]
            return np.ascontiguousarray(
                w.reshape(8, 128, -1).transpose(1, 0, 2)).astype(dt)
        def wotile(w):  # [H, C] -> [128, NCT, C]
            return np.ascontiguousarray(
                w.reshape(NCT, 128, -1).transpose(1, 0, 2)).astype(NP_BF16)
        rw = RSW if R_FP8 else 1.0
        rdt = NP_FP8 if R_FP8 else NP_BF16
        im = {
            "xT": xtile_f.astype(NP_BF16),
            "Wrf": wtile(rw * rkv_w[0 * C:1 * C][cs].T.astype(f32), rdt),
            "Wkf": wtile(rkv_w[1 * C:2 * C][cs].T.astype(f32), NP_BF16),
            "Wvf": wtile(rkv_w[2 * C:3 * C][cs].T.astype(f32), NP_BF16),
            "Wrb": wtile(rw * rkv_w[3 * C:4 * C][cs].T.astype(f32), rdt),
            "Wkb": wtile(rkv_w[4 * C:5 * C][cs].T.astype(f32), NP_BF16),
            "Wvb": wtile(rkv_w[5 * C:6 * C][cs].T.astype(f32), NP_BF16),
            "Wof": wotile((0.5 * out_w[:, cs].T).astype(f32)),
            "Wob": wotile((0.5 * out_w[:, C:][:, cs].T).astype(f32)),
        }
        if R_FP8:
            im["xT8"] = xtile_f.astype(NP_FP8)
        for nm, lam_full, eu_full in (
                ("f", lam_full_f, eu_full_f),
                ("b", lam_full_b, eu_full_b)):
            lam_loc = lam_full[cs]    # [H]
            eu_loc = eu_full[cs]
            lam_tile = np.empty((128, NCT), f32)
            eu_tile = np.empty((128, NCT), f32)
            for ct in range(NCT):
                lam_tile[:, ct] = lam_loc[ct * 128:(ct + 1) * 128]
                eu_tile[:, ct] = eu_loc[ct * 128:(ct + 1) * 128]
            im["lam" + nm] = lam_tile
            im["eu" + nm] = eu_tile
        in_maps.append(im)
    return in_maps


def run(inputs, trace=False, tmpdir=None):
    global _compiled
    if _compiled is None:
        _compiled = _build()
    in_maps = _prep_inputs(**inputs)
    tcores = None
    if os.environ.get("BIRWKV_TRACE_ALL"):
        tcores = list(range(8))
    res = run_bass_kernel_spmd(_compiled, in_maps, list(range(8)),
                               trace=trace, tmpdir=tmpdir, trace_cores=tcores)
    out = np.zeros((B, T, C), np.float32)
    for core in range(8):
        b = core // 2
        r = res.results[core]
        # [128, 8, T] -> [C, T] with c = et*128 + p
        of = r["outTf"].astype(np.float32).transpose(1, 0, 2).reshape(C, T)
        ob = r["outTb"].astype(np.float32).transpose(1, 0, 2).reshape(C, T)
        out[b] += of.T
        out[b] += ob.T[::-1]
    return out, res


def kernel(**inputs):
    out, _ = run(inputs)
    return out


# revision 56
# speedup vs baseline: 1.0100x; 1.0100x over previous
"""BiRWKV layer kernel for Trainium2 (8 NeuronCores, Bass/Tile).

Problem: x[4,2048,1024] -> rkv = x @ rkv_w.T -> (r,k,v) fwd + bwd,
WKV scan per direction, gate with sigmoid(r), concat, out @ out_w.T.

Strategy (v2):
  - Shard over (batch b, channel-half h): core = 2*b + h. Each core handles
    one batch's 512 fwd + 512 bwd channels end-to-end.
  - Channels-on-partitions layout [c, t]: projections via PE matmul
    (lhsT = W^T tile [c,d], rhs = x^T [c,t]), WKV recurrence via the DVE's
    native tensor_tensor_scan, out-projection partial via PE (contraction
    over local c), summed across cores on host (bf16 partials).
  - k,v matmuls in bf16; r matmuls in fp8e4 DoubleRow mode (2x PE rate,
    K=256 per instruction). r only passes through sigmoid (bounded
    derivative), so fp8 error stays within the rel-err budget. Wr is
    pre-scaled x32 on host; descale folded into the tanh activation scale.
  - WKV scan runs unstabilized in fp32: A_t = lam*A + e^k v, D_t = lam*D
    + e^k. Scans are reformulated as the SHIFTED recurrence
    S_t = lam*S_{t-1} + u_{t-1} (u = pv or p, with the chunk carry riding
    in slot 0 of a [128, TCH+1] input tile), so S_t = A_{t-1} lands in an
    ALIGNED [128, TCH] output that num/den read directly:
       num = eu*pv + S_A,  den = eu*p + S_D  (stt ops)
    The next chunk's carry A_TCH = lam*S_TCH + u_TCH is a tiny [128,1]
    stt on gpsimd into the next chunk's input slot 0.
  - sigmoid(r)*y = 0.5*(1+tanh(r/2))*y; the 0.5 is folded into out_w.
  - Engine placement per chain: ACT exp/tanh/v-drain + most output drains;
    DVE pv, both scans, num/den, reciprocal; gpsimd y, z, carries. Stages
    are software-pipelined with lags (rec/y lag one chain, z two) so no
    strict-FIFO queue blocks at its head.
  - Tail: the last chunk's out-projection is emitted in K-split waves
    (ct01 -> ct2 -> ct3) so the PE keeps running while the final z tiles
    come down the vector pipeline.
"""
import os
import sys
import numpy as np

sys.path.insert(0, "/opt/trn_rl_repo")

import ml_dtypes

import concourse.bass as bass
import concourse.mybir as mybir
from concourse import bacc
import concourse.tile as tile
from concourse.bass_utils import run_bass_kernel_spmd

B, T, C = 4, 2048, 1024
H = C // 2          # channels per core per direction (512)
NCT = H // 128      # c-tiles per direction (4)
TCH = 512           # time chunk
NTC = T // TCH      # t-chunks (4)
PAD = 15            # carry slot index in u-tiles; keeps p/pv reads 32B-aligned
F32 = mybir.dt.float32
BF16 = mybir.dt.bfloat16
FP8 = mybir.dt.float8e4
AF = mybir.ActivationFunctionType
ALU = mybir.AluOpType
PM = mybir.MatmulPerfMode

R_FP8 = os.environ.get("BIRWKV_R_FP8", "1") == "1"
OUT_BF16 = os.environ.get("BIRWKV_OUT_BF16", "1") == "1"
TAIL_SPLIT = os.environ.get("BIRWKV_TAIL", "1") == "1"
RSW = 32.0           # host pre-scale on Wr when fp8

NP_BF16 = ml_dtypes.bfloat16
NP_FP8 = ml_dtypes.float8_e4m3
OUT_DT = BF16 if OUT_BF16 else F32
NP_OUT = NP_BF16 if OUT_BF16 else np.float32

_compiled = None


def _build():
    nc = bacc.Bacc("TRN2", target_bir_lowering=False, debug=False, num_devices=8)

    # per-core inputs, all partition-major so batched DMAs walk source and
    # dest in the same order: xT[p, ck, t] holds x^T channel c = ck*128+p.
    xT = nc.dram_tensor("xT", [128, 8, T], BF16, kind="ExternalInput").ap()
    if R_FP8:
        xT8 = nc.dram_tensor("xT8", [128, 8, T], FP8, kind="ExternalInput").ap()
    Ws = {}
    for nm in ("Wkf", "Wvf", "Wkb", "Wvb"):
        Ws[nm] = nc.dram_tensor(nm, [128, 8, H], BF16, kind="ExternalInput").ap()
    rdt = FP8 if R_FP8 else BF16
    for nm in ("Wrf", "Wrb"):
        Ws[nm] = nc.dram_tensor(nm, [128, 8, H], rdt, kind="ExternalInput").ap()
    Wof = nc.dram_tensor("Wof", [128, NCT, C], BF16, kind="ExternalInput").ap()
    Wob = nc.dram_tensor("Wob", [128, NCT, C], BF16, kind="ExternalInput").ap()
    lamf = nc.dram_tensor("lamf", [128, NCT], F32, kind="ExternalInput").ap()
    lamb = nc.dram_tensor("lamb", [128, NCT], F32, kind="ExternalInput").ap()
    euf = nc.dram_tensor("euf", [128, NCT], F32, kind="ExternalInput").ap()
    eub = nc.dram_tensor("eub", [128, NCT], F32, kind="ExternalInput").ap()

    # outputs [p, et, t]: channel c = et*128 + p
    outTf = nc.dram_tensor("outTf", [128, 8, T], OUT_DT, kind="ExternalOutput").ap()
    outTb = nc.dram_tensor("outTb", [128, 8, T], OUT_DT, kind="ExternalOutput").ap()

    with tile.TileContext(nc) as tc:
        with (
            tc.tile_pool(name="wk", bufs=1) as wk_pool,
            tc.tile_pool(name="wv", bufs=1) as wv_pool,
            tc.tile_pool(name="wr", bufs=1) as wr_pool,
            tc.tile_pool(name="wo", bufs=1) as wo_pool,
            tc.tile_pool(name="xr", bufs=1) as xr_pool,
            tc.tile_pool(name="lam", bufs=1) as lam_pool,
            tc.tile_pool(name="ew", bufs=2) as ew_pool,
            tc.tile_pool(name="zs", bufs=4) as z_pool,
            tc.tile_pool(name="ab", bufs=2) as ab_pool,
            tc.tile_pool(name="osb", bufs=2) as osb_pool,
            tc.tile_pool(name="pp", bufs=5, space="PSUM") as pp,
            tc.tile_pool(name="po", bufs=3, space="PSUM") as po,
        ):
            # ---- resident tiles; startup-critical DMAs first -----------
            # x chunk 0 + Wkf split per-ck so their semaphores fire
            # progressively and the first matmul chain starts early.
            x_t = {}
            for q in range(8):
                x_t[(0, q)] = xr_pool.tile([128, 1, TCH], BF16,
                                           tag=f"x0q{q}", name=f"x0q{q}")
            for ti in range(1, NTC):
                x_t[ti] = xr_pool.tile([128, 8, TCH], BF16, tag=f"x{ti}",
                                       name=f"x{ti}")
            wk0q = [wk_pool.tile([128, 1, H], BF16, tag=f"wk0q{q}",
                                 name=f"wk0q{q}") for q in range(8)]
            wk_t = {1: wk_pool.tile([128, 8, H], BF16, tag="wk1", name="wk1")}
            wv_t = {d: wv_pool.tile([128, 8, H], BF16, tag=f"wv{d}",
                                    name=f"wv{d}") for d in range(2)}
            wr_t = {d: wr_pool.tile([128, 8, H], rdt, tag=f"wr{d}",
                                    name=f"wr{d}") for d in range(2)}
            wo_t = {d: wo_pool.tile([128, NCT, C], BF16, tag=f"wo{d}",
                                    name=f"wo{d}") for d in range(2)}
            x8_t = {}
            if R_FP8:
                for ti in range(NTC):
                    x8_t[ti] = xr_pool.tile([128, 8, TCH], FP8, tag=f"x8{ti}",
                                            name=f"x8{ti}")
            for q in range(8):
                nc.sync.dma_start(wk0q[q][:], Ws["Wkf"][:, q:q + 1])
                nc.sync.dma_start(x_t[(0, q)][:], xT[:, q:q + 1, 0:TCH])
            nc.sync.dma_start(wv_t[0][:], Ws["Wvf"][:])
            nc.sync.dma_start(wr_t[0][:], Ws["Wrf"][:])
            if R_FP8:
                nc.sync.dma_start(x8_t[0][:], xT8[:, :, 0:TCH])
            nc.sync.dma_start(x_t[1][:], xT[:, :, TCH:2 * TCH])
            lam_t = {}
            eu_t = {}
            for d, (lam_d, eu_d) in enumerate(((lamf, euf), (lamb, eub))):
                lt = lam_pool.tile([128, NCT], F32, tag=f"lam{d}", name=f"lt{d}")
                nc.sync.dma_start(lt[:], lam_d[:])
                et = lam_pool.tile([128, NCT], F32, tag=f"eu{d}", name=f"et{d}")
                nc.sync.dma_start(et[:], eu_d[:])
                lam_t[d] = lt
                eu_t[d] = et

            # deferred input DMAs: issued at chain checkpoints (n -> fn)
            def dma_later():
                if R_FP8:
                    yield lambda: nc.sync.dma_start(x8_t[1][:],
                                                    xT8[:, :, TCH:2 * TCH])
                yield lambda: nc.sync.dma_start(wo_t[0][:], Wof[:])
                yield lambda: nc.sync.dma_start(x_t[2][:],
                                                xT[:, :, 2 * TCH:3 * TCH])
                yield lambda: nc.sync.dma_start(x_t[3][:],
                                                xT[:, :, 3 * TCH:4 * TCH])
                if R_FP8:
                    yield lambda: nc.sync.dma_start(x8_t[2][:],
                                                    xT8[:, :, 2 * TCH:3 * TCH])
                    yield lambda: nc.sync.dma_start(x8_t[3][:],
                                                    xT8[:, :, 3 * TCH:4 * TCH])
                yield lambda: nc.sync.dma_start(wk_t[1][:], Ws["Wkb"][:])
                yield lambda: nc.sync.dma_start(wv_t[1][:], Ws["Wvb"][:])
                yield lambda: nc.sync.dma_start(wr_t[1][:], Ws["Wrb"][:])
                yield lambda: nc.sync.dma_start(wo_t[1][:], Wob[:])
            dma_q = dma_later()

            prevA = {}
            prevD = {}
            records = []
            z_by_chunk = {}
            ostages = {}
            op_q = []             # pending out-projection (chunk, e0, e1)

            def stage1(d, ti, ct, k_ps, v_ps, r_ps):
                """Baseline scan layout: p/pv in aligned full tiles (fast
                stt reads); inclusive scan A_t writes a_buf[:, 1:TCH+1]
                (initial = carry at [:, 0:1]); num/den read [:, 0:TCH]."""
                p = ew_pool.tile([128, TCH], BF16, tag="p", bufs=4, name="p")
                nc.scalar.activation(p[:], k_ps[:], AF.Exp)
                # tanh shares the ACT table set with exp (sigmoid does not);
                # z = (tanh(r/2)+1)*y with the 0.5 folded into Wo. th1 = th+1
                # is a second ACT op (Copy w/ bias) so z is ONE Pool multiply
                # off the latency-critical DVE queue.
                th = ew_pool.tile([128, TCH], BF16, tag="th", bufs=3, name="th")
                th_scale = 0.5 / RSW if R_FP8 else 0.5
                nc.scalar.activation(th[:], r_ps[:], AF.Tanh, scale=th_scale)
                v_sb = ew_pool.tile([128, TCH], BF16, tag="vsb", bufs=4,
                                    name="vsb")
                nc.scalar.copy(v_sb[:], v_ps[:])
                pv = ew_pool.tile([128, TCH], BF16, tag="pv", bufs=2,
                                  name="pv")
                nc.vector.tensor_tensor(pv[:], p[:], v_sb[:], ALU.mult)

                # 520-wide alloc keeps the partition pitch 32B-aligned
                a_buf = ab_pool.tile([128, TCH + 8], F32, tag=f"A{ct}",
                                     name="ab")
                d_buf = ab_pool.tile([128, TCH + 8], F32, tag=f"D{ct}",
                                     name="db")
                if ti == 0:
                    nc.vector.memset(a_buf[:, 0:1], 0.0)
                    nc.vector.memset(d_buf[:, 0:1], 0.0)
                else:
                    nc.gpsimd.tensor_copy(a_buf[:, 0:1],
                                          prevA[ct][:, TCH:TCH + 1])
                    nc.gpsimd.tensor_copy(d_buf[:, 0:1],
                                          prevD[ct][:, TCH:TCH + 1])
                lam_sl = lam_t[d][:, ct:ct + 1].broadcast_to([128, TCH])
                nc.vector.tensor_tensor_scan(
                    a_buf[:, 1:TCH + 1], lam_sl, pv[:],
                    a_buf[:, 0:1], ALU.mult, ALU.add)
                nc.vector.tensor_tensor_scan(
                    d_buf[:, 1:TCH + 1], lam_sl, p[:],
                    d_buf[:, 0:1], ALU.mult, ALU.add)
                prevA[ct] = a_buf
                prevD[ct] = d_buf

                num = ew_pool.tile([128, TCH], F32, tag="num", bufs=3,
                                   name="num")
                nc.vector.scalar_tensor_tensor(
                    num[:], pv[:], eu_t[d][:, ct:ct + 1], a_buf[:, 0:TCH],
                    ALU.mult, ALU.add)
                den = ew_pool.tile([128, TCH], F32, tag="den", bufs=3,
                                   name="den")
                nc.vector.scalar_tensor_tensor(
                    den[:], p[:], eu_t[d][:, ct:ct + 1], d_buf[:, 0:TCH],
                    ALU.mult, ALU.add)
                return {"d": d, "ti": ti, "ct": ct, "num": num, "den": den,
                        "th": th}

            def stage23(r, on_dve=False):
                if "y" in r:
                    return
                rec = ew_pool.tile([128, TCH], F32, tag="rec", bufs=3, name="rc")
                nc.vector.reciprocal_approx_fast(rec[:], r["den"][:])
                y = ew_pool.tile([128, TCH], BF16, tag="y", bufs=4, name="y")
                if on_dve:
                    nc.vector.tensor_mul(y[:], r["num"][:], rec[:])
                else:
                    nc.gpsimd.tensor_mul(y[:], r["num"][:], rec[:])
                r["y"] = y
                # th1 = th+1 emitted here (lag 1): off the exp->pv ACT
                # latency path, still a round ahead of its z consumer
                th1 = ew_pool.tile([128, TCH], BF16, tag="th1", bufs=3,
                                   name="th1")
                nc.scalar.activation(th1[:], r["th"][:], AF.Copy, bias=1.0)
                r["th1"] = th1

            def stage4(r, on_dve=False):
                if r.get("z_done"):
                    return
                r["z_done"] = True
                z = z_pool.tile([128, TCH], BF16, tag=f'z{r["ct"]}', name="z")
                eng = nc.vector if on_dve else nc.gpsimd
                eng.tensor_mul(z[:], r["th1"][:], r["y"][:])
                z_by_chunk[(r["d"], r["ti"])][r["ct"]] = z

            def emit_outproj(chunk, e0, e1, final=False):
                d, ti = chunk
                outT = outTb if d == 1 else outTf
                t0 = ti * TCH
                ci = d * NTC + ti
                if ci not in ostages:
                    ostages[ci] = osb_pool.tile([128, 8, TCH], OUT_DT,
                                                tag="ost", name="ost")
                ostage = ostages[ci]
                z_tiles = z_by_chunk[chunk]
                for et in range(e0, e1):
                    esl = slice(et * 128, (et + 1) * 128)
                    o_ps = po.tile([128, TCH], F32, tag="ops", name="op")
                    for ct in range(NCT):
                        nc.tensor.matmul(
                            o_ps[:],
                            wo_t[d][:, ct, esl],
                            z_tiles[ct][:],
                            start=(ct == 0), stop=(ct == NCT - 1),
                        )
                    if et % 2 == 1 and final:
                        nc.vector.tensor_copy(ostage[:, et], o_ps[:])
                    else:
                        nc.scalar.copy(ostage[:, et], o_ps[:])
                    if final:
                        nc.sync.dma_start(outT[:, et, t0:t0 + TCH],
                                          ostage[:, et])
                    elif et == 7:
                        nc.sync.dma_start(outT[:, :, t0:t0 + TCH], ostage[:])

            def emit_outproj_tail(chunk):
                """Final chunk out-proj in K-split waves: ct01 for all ets
                first (z2/z3 still in flight), then ct2, then ct3+stop."""
                d, ti = chunk
                outT = outTb if d == 1 else outTf
                t0 = ti * TCH
                ostage = osb_pool.tile([128, 8, TCH], OUT_DT, tag="ost",
                                       name="ost")
                z_tiles = z_by_chunk[chunk]
                NP = 5  # ets done via partial waves (pp ring); rest full
                parts = []
                for et in range(NP):
                    o_ps = pp.tile([128, TCH], F32, tag="proj", name="pt")
                    for ct in (0, 1):
                        nc.tensor.matmul(
                            o_ps[:], wo_t[d][:, ct, et * 128:(et + 1) * 128],
                            z_tiles[ct][:], start=(ct == 0), stop=False,
                            skip_group_check=True)
                    parts.append(o_ps)
                for et in range(NP):
                    nc.tensor.matmul(
                        parts[et][:], wo_t[d][:, 2, et * 128:(et + 1) * 128],
                        z_tiles[2][:], start=False, stop=False,
                        skip_group_check=True)
                for et in range(NP):
                    nc.tensor.matmul(
                        parts[et][:], wo_t[d][:, 3, et * 128:(et + 1) * 128],
                        z_tiles[3][:], start=False, stop=True,
                        skip_group_check=True)
                    # all tail drains on ACT: DVE is saturated at the drain
                    # (gpsimd can't read PSUM)
                    nc.scalar.copy(ostage[:, et], parts[et][:])
                    nc.sync.dma_start(outT[:, et, t0:t0 + TCH], ostage[:, et])
                for et in range(NP, 8):
                    o_ps = po.tile([128, TCH], F32, tag="ops", name="op")
                    for ct in range(NCT):
                        nc.tensor.matmul(
                            o_ps[:], wo_t[d][:, ct, et * 128:(et + 1) * 128],
                            z_tiles[ct][:],
                            start=(ct == 0), stop=(ct == NCT - 1))
                    nc.scalar.copy(ostage[:, et], o_ps[:])
                    nc.sync.dma_start(outT[:, et, t0:t0 + TCH], ostage[:, et])

            def wsl(d, w, ck, dsl):
                if w == "k" and d == 0:
                    return wk0q[ck][:, 0, dsl]
                wmap = {"k": wk_t.get(d), "v": wv_t[d], "r": wr_t[d]}
                return wmap[w][:, ck, dsl]

            def xsl(tis, ck, rev):
                if tis == 0:
                    s = x_t[(0, ck)][:, 0]
                else:
                    s = x_t[tis][:, ck]
                return s[:, ::-1] if rev else s

            n = 0
            for d in range(2):
                rev = (d == 1)
                prevA.clear()
                prevD.clear()
                for ti in range(NTC):
                    z_by_chunk[(d, ti)] = [None] * NCT
                    tis = NTC - 1 - ti if rev else ti
                    # chunk 0: lead with k-sweeps (weights stream in per-ck)
                    # but keep <=5 live psum tiles (pp ring is 5)
                    order = ([("k", 0), ("k", 1), ("k", 2), ("v", 0),
                              ("r", 0), ("k", 3), ("v", 1), ("r", 1),
                              ("v", 2), ("r", 2), ("v", 3), ("r", 3)]) \
                        if (d == 0 and ti == 0) else \
                        [(w, ct) for ct in range(NCT) for w in ("k", "v", "r")]
                    kvr_ps = {}
                    for w, ct in order:
                        dsl = slice(ct * 128, (ct + 1) * 128)
                        ps = pp.tile([128, TCH], F32, tag="proj", name="ps")
                        if w == "r" and R_FP8:
                            for qq in range(4):
                                x8s = x8_t[tis][:, 2 * qq:2 * qq + 2]
                                if rev:
                                    x8s = x8s[:, :, ::-1]
                                nc.tensor.matmul(
                                    ps[:], wr_t[d][:, 2 * qq:2 * qq + 2, dsl],
                                    x8s,
                                    start=(qq == 0), stop=(qq == 3),
                                    perf_mode=PM.DoubleRow,
                                )
                        else:
                            for ck in range(8):
                                nc.tensor.matmul(
                                    ps[:], wsl(d, w, ck, dsl),
                                    xsl(tis, ck, rev),
                                    start=(ck == 0), stop=(ck == 7),
                                )
                        kvr_ps[(w, ct)] = ps
                        if w != "r":
                            continue
                        # deferred DMA issue, spread across early chains
                        if n < 10:
                            fn = next(dma_q, None)
                            if fn is not None:
                                fn()
                        # out-projection pop FIRST: its psum drain lands in
                        # the ACT queue ahead of this chain's 4 ACT ops.
                        # Only pop once all four z tiles exist (the (1,2)
                        # boundary round would otherwise race the collapse).
                        if n >= 10 and op_q:
                            cj, e0, e1, mn = op_q[0]
                            if n >= mn and all(
                                    zt is not None for zt in z_by_chunk[cj]):
                                op_q.pop(0)
                                emit_outproj(cj, e0, e1)
                        # staged vchain emission: consumers of last-round
                        # outputs first so no FIFO blocks at its head
                        if n >= 1:
                            stage23(records[n - 1])
                        if n >= 2:
                            stage4(records[n - 2])
                        records.append(stage1(d, ti, ct,
                                              kvr_ps[("k", ct)],
                                              kvr_ps[("v", ct)],
                                              kvr_ps[("r", ct)]))
                        # backward direction: collapse stage lags so z tiles
                        # enqueue as early as possible (Pool is shallow)
                        if d == 1:
                            stage23(records[n])
                            if n >= 1:
                                stage4(records[n - 1])
                        # out-projection: chunk j enqueues its three et
                        # groups once its z tiles are in flight; the PE pops
                        # at most one group per chain starting at chain 10
                        if ct == 3 and not (d == 1 and ti == NTC - 1):
                            cj = records[n - 3]["d"], records[n - 3]["ti"]
                            mb = n - 3 + (7 if d == 0 else 10)
                            op_q.extend([(cj, 0, 2, mb), (cj, 2, 4, mb + 1),
                                         (cj, 4, 8, mb + 2)])
                        n += 1

            # drain: finish pipeline, flush pending emissions, then the
            # final chunk (K-split waves cover its z latency)
            stage23(records[n - 1])
            stage4(records[n - 2])
            stage4(records[n - 1])
            for cj, e0, e1, _mn in op_q:
                emit_outproj(cj, e0, e1)
            if TAIL_SPLIT:
                emit_outproj_tail((1, NTC - 1))
            else:
                emit_outproj((1, NTC - 1), 0, 8, final=True)

    nc.compile()
    return nc


def _prep_inputs(x, rkv_w, out_w, time_decay, time_first, time_decay_rev,
                 time_first_rev):
    """Host-side sharding + layout prep. Returns list of 8 input dicts."""
    f32 = np.float32
    in_maps = []
    wd_f = -np.exp(time_decay.astype(np.float64))
    wd_b = -np.exp(time_decay_rev.astype(np.float64))
    lam_full_f = np.exp(wd_f).astype(f32)        # [C]
    lam_full_b = np.exp(wd_b).astype(f32)
    eu_full_f = np.exp(time_first.astype(np.float64)).astype(f32)
    eu_full_b = np.exp(time_first_rev.astype(np.float64)).astype(f32)

    for core in range(8):
        b, h = core // 2, core % 2
        cs = slice(h * H, h * H + H)
        xb = x[b].T.astype(f32)                                    # [C, T]
        xtile_f = np.ascontiguousarray(
            xb.reshape(8, 128, T).transpose(1, 0, 2))
        def wtile(w, dt):   # [C, H] -> [128, 8, H]
            return np.ascontiguousarray(
                w.reshape(8, 128, -1).transpose(1, 0, 2)).astype(dt)
        def wotile(w):  # [H, C] -> [128, NCT, C]
            return np.ascontiguousarray(
                w.reshape(NCT, 128, -1).transpose(1, 0, 2)).astype(NP_BF16)
        rw = RSW if R_FP8 else 1.0
        rdt = NP_FP8 if R_FP8 else NP_BF16
        im = {
            "xT": xtile_f.astype(NP_BF16),
            "Wrf": wtile(rw * rkv_w[0 * C:1 * C][cs].T.astype(f32), rdt),
            "Wkf": wtile(rkv_w[1 * C:2 * C][cs].T.astype(f32), NP_BF16),
            "Wvf": wtile(rkv_w[2 * C:3 * C][cs].T.astype(f32), NP_BF16),
            "Wrb": wtile(rw * rkv_w[3 * C:4 * C][cs].T.astype(f32), rdt),
            "Wkb": wtile(rkv_w[4 * C:5 * C][cs].T.astype(f32), NP_BF16),
            "Wvb": wtile(rkv_w[5 * C:6 * C][cs].T.astype(f32), NP_BF16),
            "Wof": wotile((0.5 * out_w[:, cs].T).astype(f32)),
            "Wob": wotile((0.5 * out_w[:, C:][:, cs].T).astype(f32)),
        }
        if R_FP8:
            im["xT8"] = xtile_f.astype(NP_FP8)
        for nm, lam_full, eu_full in (
                ("f", lam_full_f, eu_full_f),
                ("b", lam_full_b, eu_full_b)):
            lam_loc = lam_full[cs]    # [H]
            eu_loc = eu_full[cs]
            lam_tile = np.empty((128, NCT), f32)
            eu_tile = np.empty((128, NCT), f32)
            for ct in range(NCT):
                lam_tile[:, ct] = lam_loc[ct * 128:(ct + 1) * 128]
                eu_tile[:, ct] = eu_loc[ct * 128:(ct + 1) * 128]
            im["lam" + nm] = lam_tile
            im["eu" + nm] = eu_tile
        in_maps.append(im)
    return in_maps


def run(inputs, trace=False, tmpdir=None):
    global _compiled
    if _compiled is None:
        _compiled = _build()
    in_maps = _prep_inputs(**inputs)
    tcores = None
    if os.environ.get("BIRWKV_TRACE_ALL"):
        tcores = list(range(8))
    res = run_bass_kernel_spmd(_compiled, in_maps, list(range(8)),
                               trace=trace, tmpdir=tmpdir, trace_cores=tcores)
    out = np.zeros((B, T, C), np.float32)
    for core in range(8):
        b = core // 2
        r = res.results[core]
        # [128, 8, T] -> [C, T] with c = et*128 + p
        of = r["outTf"].astype(np.float32).transpose(1, 0, 2).reshape(C, T)
        ob = r["outTb"].astype(np.float32).transpose(1, 0, 2).reshape(C, T)
        out[b] += of.T
        out[b] += ob.T[::-1]
    return out, res


def kernel(**inputs):
    out, _ = run(inputs)
    return out
